# revision 65
# baseline (speedup 1.0000x reference)
"""Trainium2 Bass kernel for nn_DocREModel (doc-level relation extraction graph pooling).

Strategy (8 NeuronCores): each doc b (B=4) is split across 2 cores.  The model
only ever reads attention rows at mention positions (EM=128) and inside link
spans (~250 rows), and span rows only matter at span COLUMNS (~250).  Each core
device-gathers (SWDGE indirect DMA, runtime per-partition index tiles):

  - mention rows from att_r [row, head*512+col] (fp8): all 12 heads of this
    core's geometric column half (6 KB/row) -> head-sum via fp8 DoubleRow
    identity matmuls -> PE transposes -> mnum = S_mention^T @ [seq|1]
    (context numerators + row-sums; partial over 512 columns, host sums pair).
  - span rows from att_sp [row, head*128+ccol] (fp8): a host-compressed column
    subset (this core takes every other span column, <=128) so each row is
    only 1.5 KB -> head-sum -> slot-mask matmul gives span row-sums
    uT[16,128] -> mask, one PE transpose back, v = wv^T @ [seqc|1]
    (partial over this core's span columns, host sums pair).
The host gathers mention embeddings (pure input row-gather) and applies the
tiny normalizations (head-count / span-length / row-sum divides, entity
pooling, 4-way logsumexp) while unsharding.
"""

import os
import sys

for _p in ("/opt/trn_rl_repo", "/root/.axon_site/_ro/trn_rl_repo"):
    if os.path.isdir(_p) and _p not in sys.path:
        sys.path.insert(0, _p)

import numpy as np

B, L, H, NH = 4, 1024, 768, 12
E, MPE, K = 32, 4, 16
EM = E * MPE              # 128 mentions per doc == the mention gather chunk
TYPE_DIM = 20
OFFSET = 1
CW = L // 2               # mnum columns per core (2 cores per doc)
CTH = CW // 128           # 4 column chunks per core (mnum path)
HA = H + 4                # seq | ones | 3 zero-pad -> 772 (row-sum in col 768)
NSP_DEF = 2               # span-row chunks (<=256 distinct span rows)
NCC_DEF = 1               # compressed span-column chunks per core (<=128 cols)


def _build_nc(nsp=NSP_DEF, ncc=NCC_DEF, debug=False):
    import concourse.bass as bass
    import concourse.mybir as mybir
    import concourse.tile as tile
    from concourse import bacc

    f32 = mybir.dt.float32
    bf16 = mybir.dt.bfloat16
    fp8 = mybir.dt.float8e4  # e4m3
    i32 = mybir.dt.int32
    u8 = mybir.dt.uint8
    ts, ds = bass.ts, bass.ds
    CC = ncc * 128            # compressed span columns per core

    # const blob layout (bytes): ident82 fp8 [0,256) | ident bf16 [256,512)
    #   | maskS_sp bf16 | idx_m i32 | idx_sp i32
    mS_off = 512
    idxm_off = mS_off + 2 * nsp * K
    idxsp_off = idxm_off + 4
    blob_w = idxsp_off + 4 * nsp

    nc = bacc.Bacc("TRN2", target_bir_lowering=False, debug=debug)

    att_r = nc.dram_tensor("att_r", [L, NH * CW], fp8, kind="ExternalInput")
    att_sp = nc.dram_tensor("att_sp", [L, NH * CC], fp8, kind="ExternalInput")
    seqb = nc.dram_tensor("seqb", [CW, HA], bf16, kind="ExternalInput")
    seqc = nc.dram_tensor("seqc", [CC, HA], bf16, kind="ExternalInput")
    cblob = nc.dram_tensor("cblob", [128, blob_w], u8, kind="ExternalInput")
    maskCT = nc.dram_tensor("maskCT", [K, CC], f32, kind="ExternalInput")
    out_mnum = nc.dram_tensor("out_mnum", [EM, HA], bf16, kind="ExternalOutput")
    out_v = nc.dram_tensor("out_v", [K, HA], bf16, kind="ExternalOutput")

    with tile.TileContext(nc) as tc:
        with (
            tc.tile_pool(name="const", bufs=1) as constp,
            tc.tile_pool(name="gat", bufs=1) as gatp,
            tc.tile_pool(name="acc", bufs=1) as accp,
            tc.tile_pool(name="stage", bufs=1) as stagep,
            tc.tile_pool(name="pshold", bufs=1, space="PSUM") as pshold,
            tc.tile_pool(name="psrot", bufs=2, space="PSUM") as psrot,
        ):
            # dependency-free dummy SWDGE DMA: pre-warms the Q7 descriptor
            # path so the real gather issues immediately once indices land
            warm_s = constp.tile([128, 64], u8, tag="swwarm", name="swwarm")
            nc.gpsimd.dma_start(out=warm_s[:], in_=cblob[:, 0:64])

            # ---- one const-blob DMA first: indices + tiny matrices together ----
            blob_s = constp.tile([128, blob_w], u8, tag="cblob", name="cblob")
            nc.sync.dma_start(out=blob_s[:], in_=cblob[:])
            idxm_s = blob_s[:, ds(idxm_off, 4)].bitcast(i32)
            idxsp_s = blob_s[:, ds(idxsp_off, 4 * nsp)].bitcast(i32)
            maskS_s = blob_s[:, ds(mS_off, 2 * nsp * K)].bitcast(bf16).rearrange(
                "p (q k) -> p q k", k=K)
            ident_s = blob_s[:, ds(256, 256)].bitcast(bf16)
            ident82_s = blob_s[:, ds(0, 256)].bitcast(fp8)

            # mention-row gather first (it gates the longest chain)
            g5m = gatp.tile([128, NH * CW], fp8, tag="g5m", name="g5m")
            nc.gpsimd.indirect_dma_start(
                out=g5m[:], out_offset=None, in_=att_r[:],
                in_offset=bass.IndirectOffsetOnAxis(ap=idxm_s[:], axis=0))
            g5sp = []
            for q in range(nsp):
                g = gatp.tile([128, NH * CC], fp8, tag="g5sp", name="g5sp", bufs=nsp)
                nc.gpsimd.indirect_dma_start(
                    out=g[:], out_offset=None, in_=att_sp[:],
                    in_offset=bass.IndirectOffsetOnAxis(ap=idxsp_s[:, q:q + 1], axis=0))
                g5sp.append(g)

            # PE warm-up while the first gather is in flight: >=3.4us of busy
            # keeps the HAM clock gate at 8/8 so the first head-sum matmuls
            # run at 2.4 GHz (16 x ~0.27us spans the gather window)
            for _ in range(16):
                phw = psrot.tile([128, 512], f32, tag="ph", name="phw")
                nc.tensor.matmul(phw[:, 0:256], ident82_s[:, 0:128],
                                 ident82_s[:], start=True, stop=True)

            # ---- remaining consts ----
            maskCT_s = constp.tile([K, CC], f32, tag="maskCT", name="maskCT")
            seq_s = constp.tile([128, CTH, HA], bf16, tag="seqs", name="seqs")
            seqc_s = constp.tile([128, ncc, HA], bf16, tag="seqc", name="seqc")
            nc.scalar.dma_start(out=maskCT_s[:], in_=maskCT[:])
            nc.scalar.dma_start(out=seq_s[:], in_=seqb[:].rearrange("(c p) f -> p c f", p=128))
            nc.scalar.dma_start(out=seqc_s[:], in_=seqc[:].rearrange("(c p) f -> p c f", p=128))

            dr_w = ident82_s.rearrange("p (two m) -> p two m", two=2)

            # ---- mention chain: head-sum, transposes, mnum ----
            gsm_s = accp.tile([128, CW], bf16, tag="gsm", name="gsm")
            gtm_s = accp.tile([128, CTH, 128], bf16, tag="gtm", name="gtm")
            g5m_r = g5m[:].rearrange("p (h c) -> p h c", h=NH)
            phm = psrot.tile([128, 512], f32, tag="ph", name="phm")
            for hp in range(NH // 2):
                nc.tensor.matmul(phm[:], dr_w, g5m_r[:, ds(2 * hp, 2), :],
                                 start=(hp == 0), stop=(hp == NH // 2 - 1),
                                 perf_mode=mybir.MatmulPerfMode.DoubleRow)
            nc.scalar.copy(out=gsm_s[:, 0:256], in_=phm[:, 0:256])
            nc.vector.tensor_copy(gsm_s[:, 256:512], phm[:, 256:512])
            # gtm copies on ACT so the DVE stays clear for the v-chain's
            # wvt multiply (otherwise it queues behind these in FIFO order)
            for ct in range(CTH):
                pt = psrot.tile([128, 128], bf16, tag="pt", name="pt")
                nc.tensor.transpose(pt[:], gsm_s[:, ts(ct, 128)], ident_s)
                nc.scalar.copy(out=gtm_s[:, ct, :], in_=pt[:])
            pm0 = pshold.tile([EM, 512], f32, tag="pm0", name="pm0")
            pm1 = pshold.tile([EM, HA - 512], f32, tag="pm1", name="pm1")
            for ct in range(CTH):
                st, sp = (ct == 0), (ct == CTH - 1)
                nc.tensor.matmul(pm0[:], gtm_s[:, ct, :], seq_s[:, ct, 0:512], start=st, stop=sp)
                nc.tensor.matmul(pm1[:], gtm_s[:, ct, :], seq_s[:, ct, 512:HA], start=st, stop=sp)
            mnum_s = stagep.tile([EM, HA], bf16, tag="mnum", name="mnum")
            nc.scalar.copy(out=mnum_s[:, 0:512], in_=pm0[:])
            nc.vector.tensor_copy(mnum_s[:, 512:HA], pm1[:])
            nc.sync.dma_start(out=out_mnum[:], in_=mnum_s[:])

            # ---- span chain: head-sum (compressed cols), uT, wv, v ----
            gsp_s = accp.tile([128, nsp, CC], bf16, tag="gsp", name="gsp")
            put = pshold.tile([K, CC], f32, tag="put", name="put")
            for q in range(nsp):
                gr = g5sp[q][:].rearrange("p (h c) -> p h c", h=NH)
                ph = psrot.tile([128, 512], f32, tag="ph", name="ph")
                for hp in range(NH // 2):
                    nc.tensor.matmul(ph[:, 0:CC], dr_w, gr[:, ds(2 * hp, 2), :],
                                     start=(hp == 0), stop=(hp == NH // 2 - 1),
                                     perf_mode=mybir.MatmulPerfMode.DoubleRow)
                nc.scalar.copy(out=gsp_s[:, q, :], in_=ph[:, 0:CC])
                nc.tensor.matmul(put[:], maskS_s[:, q, :], gsp_s[:, q, :],
                                 start=(q == 0), stop=(q == nsp - 1))
            wvt_s = accp.tile([K, CC], bf16, tag="wvt", name="wvt")
            wv_s = accp.tile([128, ncc, K], bf16, tag="wv", name="wv")
            nc.vector.tensor_mul(wvt_s[:], put[:], maskCT_s[:])
            for cc in range(ncc):
                ptk = psrot.tile([128, 128], bf16, tag="pt", name="ptk")
                nc.tensor.transpose(ptk[:, 0:K], wvt_s[:, ts(cc, 128)], ident_s[0:K, 0:K])
                nc.vector.tensor_copy(wv_s[:, cc, :], ptk[:, 0:K])
            pv0 = psrot.tile([128, 512], f32, tag="ph", name="pv0")
            pv1 = psrot.tile([128, 512], f32, tag="ph", name="pv1")
            for cc in range(ncc):
                st, sp = (cc == 0), (cc == ncc - 1)
                nc.tensor.matmul(pv0[0:K, :], wv_s[:, cc, :], seqc_s[:, cc, 0:512],
                                 start=st, stop=sp)
                nc.tensor.matmul(pv1[0:K, 0:HA - 512], wv_s[:, cc, :],
                                 seqc_s[:, cc, 512:HA], start=st, stop=sp)
            v_s = stagep.tile([K, HA], bf16, tag="v", name="v")
            nc.scalar.copy(out=v_s[:, 0:512], in_=pv0[0:K, :])
            nc.vector.tensor_copy(v_s[:, 512:HA], pv1[0:K, 0:HA - 512])
            nc.scalar.dma_start(out=out_v[:], in_=v_s[:])

    nc.compile()
    return nc


_NC_CACHE = {}


def _get_nc(nsp=NSP_DEF, ncc=NCC_DEF):
    if (nsp, ncc) not in _NC_CACHE:
        _NC_CACHE[(nsp, ncc)] = _build_nc(nsp, ncc)
    return _NC_CACHE[(nsp, ncc)]


def _per_core_inputs(sequence_output, attention, mention_pos, link_start, link_len):
    """Returns (in_maps for 8 cores, per-doc span lengths, per-doc membs, nsp, ncc)."""
    import ml_dtypes
    seq = np.ascontiguousarray(np.asarray(sequence_output, dtype=np.float32))
    att = np.asarray(attention)
    mpos = np.asarray(mention_pos).astype(np.int64)
    lstart = np.asarray(link_start).astype(np.int64)
    llen = np.asarray(link_len).astype(np.int64)

    docs = []
    max_sp = 0
    max_cc = 0
    for b in range(B):
        pos = (mpos[b] + OFFSET).reshape(EM)
        s = lstart[b] + OFFSET
        e = lstart[b] + llen[b] + 1 + OFFSET
        sprows = np.unique(np.concatenate(
            [np.arange(int(si), int(ei)) for si, ei in zip(s, e)]))
        docs.append((pos, s, e, sprows))
        max_sp = max(max_sp, len(sprows))
        max_cc = max(max_cc, len(sprows[0::2]), len(sprows[1::2]))
    nsp = max(NSP_DEF, -(-max_sp // 128))
    ncc = max(NCC_DEF, -(-max_cc // 128))
    CC = ncc * 128

    ident_b = np.eye(128, dtype=ml_dtypes.bfloat16).view(np.uint8)
    i8 = np.eye(128, dtype=ml_dtypes.float8_e4m3fn)
    ident82_b = np.concatenate([i8, i8], axis=1).view(np.uint8)
    in_maps = []
    lengths = []
    membs = []
    for b in range(B):
        pos, s, e, sprows = docs[b]
        lengths.append((e - s).astype(np.float32))
        membs.append(seq[b][pos])  # mention embeddings: pure input row-gather

        sl_sp = np.zeros(nsp * 128, np.int32)
        sl_sp[:len(sprows)] = sprows
        mS = np.zeros((nsp * 128, K), np.float32)
        spset = {int(r): j for j, r in enumerate(sprows)}
        for k, (si, ei) in enumerate(zip(s, e)):
            for r in range(int(si), int(ei)):
                mS[spset[r], k] = 1.0
        maskS_b = np.ascontiguousarray(
            mS.reshape(nsp, 128, K).transpose(1, 0, 2).reshape(128, nsp * K)
        ).astype(ml_dtypes.bfloat16).view(np.uint8)
        idxm_b = np.ascontiguousarray(pos.astype(np.int32).reshape(128, 1)).view(np.uint8)
        idxsp_b = np.ascontiguousarray(sl_sp.reshape(nsp, 128).T).view(np.uint8)
        cblob = np.ascontiguousarray(np.concatenate(
            [ident82_b, ident_b, maskS_b, idxm_b, idxsp_b], axis=1))
        seqb_full = np.concatenate(
            [seq[b], np.ones((L, 1), np.float32), np.zeros((L, HA - H - 1), np.float32)],
            axis=1).astype(ml_dtypes.bfloat16)
        att8 = att[b].astype(ml_dtypes.float8_e4m3fn)  # [12, L, L]
        for g in range(2):
            cols = slice(g * CW, (g + 1) * CW)
            ccols = sprows[g::2]                       # interleaved span-col split
            ccols_p = np.zeros(CC, np.int64)
            ccols_p[:len(ccols)] = ccols
            mCT = np.zeros((K, CC), np.float32)
            for k, (si, ei) in enumerate(zip(s, e)):
                mCT[k, :len(ccols)] = ((ccols >= si) & (ccols < ei))
            att_r = np.ascontiguousarray(
                att8[:, :, cols].transpose(1, 0, 2).reshape(L, NH * CW))
            att_sp = np.ascontiguousarray(
                att8[:, :, ccols_p].transpose(1, 0, 2).reshape(L, NH * CC))
            in_maps.append({
                "att_r": att_r, "att_sp": att_sp,
                "seqb": np.ascontiguousarray(seqb_full[cols]),
                "seqc": np.ascontiguousarray(seqb_full[ccols_p]),
                "cblob": cblob, "maskCT": mCT,
            })
    return in_maps, lengths, membs, nsp, ncc


def _combine(outs, lengths, membs, type_table):
    ttab = np.asarray(type_table, dtype=np.float32)
    type_ids = np.concatenate(
        [np.zeros(E, np.int64), np.ones(EM, np.int64), np.full(K, 2, np.int64)])
    nodes_type = ttab[type_ids]  # [E+EM+K, TYPE_DIM]

    out = np.zeros((B, E + EM + K + E + EM, H + TYPE_DIM), np.float32)
    for b in range(B):
        o0, o1 = outs[2 * b], outs[2 * b + 1]
        v = np.asarray(o0["out_v"], np.float32) + np.asarray(o1["out_v"], np.float32)
        mnum = (np.asarray(o0["out_mnum"], np.float32)
                + np.asarray(o1["out_mnum"], np.float32))
        memb = membs[b]
        length = lengths[b]

        link_rep = v[:, :H] / (NH * length[:, None])
        m_ctx = mnum[:, :H] / (mnum[:, H:H + 1] + NH * 1e-5)
        enum = mnum.reshape(E, MPE, HA).sum(axis=1)
        e_ctx = enum[:, :H] / (enum[:, H:H + 1] + NH * MPE * 1e-5)

        mg = memb.reshape(E, MPE, H)
        mmax = mg.max(axis=1)
        eemb = np.log(np.exp(mg - mmax[:, None, :]).sum(axis=1)) + mmax

        nodes_raw = np.concatenate([eemb, memb, link_rep], axis=0)      # [176,H]
        nodes = np.concatenate([nodes_raw, nodes_type], axis=1)         # [176,H+20]
        ctx = np.concatenate([e_ctx, m_ctx], axis=0)                    # [160,H]
        ctx = np.concatenate([ctx, np.zeros((E + EM, TYPE_DIM), np.float32)], axis=1)
        out[b] = np.concatenate([nodes, ctx], axis=0)
    return out


def kernel(**inputs):
    from concourse.bass_utils import run_bass_kernel_spmd

    in_maps, lengths, membs, nsp, ncc = _per_core_inputs(
        inputs["sequence_output"], inputs["attention"],
        inputs["mention_pos"], inputs["link_start"], inputs["link_len"])
    nc = _get_nc(nsp, ncc)
    res = run_bass_kernel_spmd(nc, in_maps, core_ids=list(range(8)))
    return _combine(res.results, lengths, membs, inputs["type_table"])
